# revision 50
# baseline (speedup 1.0000x reference)
"""GQA causal-attention prefill kernel for 8 Trainium2 NeuronCores.

Sharding: core c -> (batch b = c//4, kv head g = c%4).
Replica groups [[0,1,2,3],[4,5,6,7]] (one per batch).

All operands bf16 (PSUM accumulation fp32); rel err ~4e-3 vs the fp32
reference (tolerance 2e-2). Per-core pipeline, everything feature-major
so the token dim is always the matmul moving dim:
  1. k^T/v^T/q^T projections (+bias) from x^T, contraction over D=3584.
     Inputs arrive host-pre-packed in SBUF layout so every DMA is one
     contiguous run per partition; the x feed is split across the
     sync/scalar queues with a small first chunk so the PE starts ~15us
     in, and the per-head wq tiles stream just-in-time on the gpsimd
     queue so they never contend with the x stream.
  2. RoPE on k then q (7 heads) on DVE (fp32 intermediates, one bf16
     rounding)
  3. v^T -> v (natural) via PE transposes (P@V needs keys on partitions)
  4. per head: S^T = k^T-chunk.T @ q^T (causal chunks only), +tri-mask on
     diagonal chunks, exp on ACT -> bf16 (no max subtraction: |logits| is
     small), denominator = ones.T @ E^T on PE, O^T_unnorm = v-chunk.T @ E^T,
     normalize via DVE reciprocal + bf16 PE broadcast matmul. One
     AllGather per head, triggered the moment the head finishes (the
     gpsimd queue holds only AG triggers, so no trigger ever queues
     behind a wave-completion wait) -- the o_proj consumer then tolerates
     ~65us of inter-core launch skew before it would stall.
  5. oag->otf gathers run wave-major on the sync queue after attention;
     o_proj accumulates per AG wave into PSUM then a SBUF accumulator.
Output per core: y[b][:, 896g:896(g+1)].T, host concatenates + transposes.
"""
import sys

if '/opt/trn_rl_repo' not in sys.path:
    sys.path.insert(0, '/opt/trn_rl_repo')

import ml_dtypes
import numpy as np

B, T, D = 2, 1024, 3584
NUM_HEADS, HEAD_DIM, NUM_KV = 28, 128, 4
REP = NUM_HEADS // NUM_KV            # 7
ROPE_THETA = 1000000.0
K_MASK = -3.3895313892515355e+38     # bf16 finfo min, as in the reference
SCALE = HEAD_DIM ** -0.5
GROUP = 4                            # tensor-parallel group size (kv heads)
NCORES = 8
DK = D // 128                        # 28 contraction chunks over D
NT = T // 512                        # token 512-tiles
SK = T // 128                        # key 128-chunks
RG = [[0, 1, 2, 3], [4, 5, 6, 7]]

_CACHE = {}


def _build_nc():
    """Build the SPMD Bass program (same program on all 8 cores)."""
    import concourse.tile as tile
    from concourse import bacc, bass_isa, mybir
    from concourse.masks import make_identity

    FP32 = mybir.dt.float32
    BF16 = mybir.dt.bfloat16
    Exp = mybir.ActivationFunctionType.Exp
    Ident = mybir.ActivationFunctionType.Identity
    mult = mybir.AluOpType.mult
    addop = mybir.AluOpType.add

    nc = bacc.Bacc("TRN2", target_bir_lowering=False, debug=False, num_devices=NCORES)

    # weights arrive pre-transposed to [partition, chunk, out] layout so each
    # DMA is one contiguous descriptor per partition (fast sequencer issue)
    xt = nc.dram_tensor("xt", [128, DK, T], BF16, kind="ExternalInput")
    wq = nc.dram_tensor("wq", [128, REP, DK, 128], BF16, kind="ExternalInput")
    wk = nc.dram_tensor("wk", [128, DK, 128], BF16, kind="ExternalInput")
    wv = nc.dram_tensor("wv", [128, DK, 128], BF16, kind="ExternalInput")
    wo = nc.dram_tensor("wo", [128, DK, REP * 128], BF16, kind="ExternalInput")
    bqkv = nc.dram_tensor("bqkv", [REP + 2, 128], FP32, kind="ExternalInput")
    sincat = nc.dram_tensor("sincat", [128, T], FP32, kind="ExternalInput")
    coscat = nc.dram_tensor("coscat", [128, T], FP32, kind="ExternalInput")
    trimask = nc.dram_tensor("trimask", [128, 128], FP32, kind="ExternalInput")
    onescol = nc.dram_tensor("onescol", [128, 1], BF16, kind="ExternalInput")
    onesrow = nc.dram_tensor("onesrow", [1, 128], BF16, kind="ExternalInput")
    yt = nc.dram_tensor("yt", [REP * 128, T], FP32, kind="ExternalOutput")

    with tile.TileContext(nc) as tc:
        with (
            tc.tile_pool(name="consts", bufs=1) as consts,
            tc.tile_pool(name="qkv", bufs=1) as qkv,
            tc.tile_pool(name="dram", bufs=1, space="DRAM") as dram,
            tc.tile_pool(name="ep", bufs=4) as ep,
        ):
            tri_sb = consts.tile([128, 128], FP32, tag="tri")
            ones_col = consts.tile([128, 1], BF16, tag="onescol")
            ones_row = consts.tile([1, 128], BF16, tag="onesrow")
            bias_sb = consts.tile([128, REP + 2], FP32, tag="bias")
            # consts go on the scalar DMA queue so they don't delay the
            # weight/x stream on the sync queue
            nc.scalar.dma_start(tri_sb[:], trimask[:])
            nc.scalar.dma_start(ones_col[:], onescol[:])
            nc.scalar.dma_start(ones_row[:], onesrow[:])
            nc.scalar.dma_start(bias_sb[:], bqkv.rearrange("m p -> p m"))

            q_sb = qkv.tile([128, REP, T], BF16, tag="q")
            k_sb = qkv.tile([128, T], BF16, tag="k")
            vn_sb = qkv.tile([128, SK, 128], BF16, tag="vn")

            # head-group DRAM blocks for the pipelined AllGather (bf16);
            # one wave per head: each AG fires as early as possible, so the
            # o_proj consumer tolerates ~65us of inter-core launch skew
            AGH = [(h, h + 1) for h in range(REP)]     # [lo, hi) head ranges
            og = [dram.tile([(hi - lo) * 128, T], BF16, tag=f"og{i}", name=f"og{i}")
                  for i, (lo, hi) in enumerate(AGH)]
            oag = [dram.tile([GROUP * (hi - lo) * 128, T], BF16,
                             tag=f"oag{i}", name=f"oag{i}")
                   for i, (lo, hi) in enumerate(AGH)]

            # warmup collective (trigger emitted in phase 1 after the weight
            # DMA issues): absorbs first-op CC-stream setup cost
            wu_in = dram.tile([128, 16], BF16, tag="wuin", name="wuin")
            wu_out = dram.tile([GROUP * 128, 16], BF16, tag="wuout", name="wuout")
            wu_sb = consts.tile([128, 16], BF16, tag="wusb")
            nc.vector.memset(wu_sb[:], 0.0)
            nc.scalar.dma_start(wu_in[:], wu_sb[:])
            nc.gpsimd.collective_compute(
                "AllGather",
                mybir.AluOpType.bypass,
                replica_groups=RG,
                ins=[wu_in[:].opt()],
                outs=[wu_out[:].opt()],
            )

            # PE clock pre-ramp: the Tensor engine starts at a low pstate
            # and only reaches full clock after ~3us of continuous
            # execution. Run a throwaway matmul chain during the initial
            # DMA wait so the real projection chain starts at full speed.
            with tc.tile_pool(name="warmp", bufs=1, space="PSUM") as warmp:
                warm = consts.tile([128, 512], BF16, tag="warm")
                nc.vector.memset(warm[:], 0.0)
                wps = warmp.tile([128, 512], FP32, tag="wps")
                for i in range(8):
                    nc.tensor.matmul(
                        wps[:], warm[:, 0:128], warm[:],
                        start=(i == 0), stop=(i == 7),
                    )

            # ---- Phase 1: projections (k, v first, then q heads) --------
            with (
                tc.tile_pool(name="xp", bufs=1) as xp,
                tc.tile_pool(name="wp", bufs=1) as wp,
                tc.tile_pool(name="vp", bufs=1) as vp,
                tc.tile_pool(name="ropep", bufs=2) as ropep,
                tc.tile_pool(name="sincosp", bufs=1) as sincosp,
                tc.tile_pool(name="pp1", bufs=2, space="PSUM") as pp1,
            ):
                sin_sb = sincosp.tile([128, T], FP32, tag="sin")
                cos_sb = sincosp.tile([128, T], FP32, tag="cos")
                id_sb = vp.tile([128, 128], BF16, tag="ident")
                make_identity(nc, id_sb[:])
                v_sb = vp.tile([128, T], BF16, tag="v")

                # DMA plan: xt arrives pre-packed in SBUF layout, loaded as
                # four 1.8MB fully-contiguous quarters interleaved across
                # the sync/scalar queues; wq heads stream just-in-time on
                # the gpsimd queue. The m=0 chain can start ~15us in and
                # never stalls on the x feed.
                wk_sb = wp.tile([128, DK, 128], BF16, tag="wk")
                wv_sb = wp.tile([128, DK, 128], BF16, tag="wv")
                nc.sync.dma_start(wk_sb[:], wk[:])

                x_sb = xp.tile([128, DK, T], BF16, tag="x")
                nc.sync.dma_start(x_sb[:, 0:4, :], xt[:, 0:4, :])
                nc.scalar.dma_start(x_sb[:, 4:11, :], xt[:, 4:11, :])
                nc.sync.dma_start(x_sb[:, 11:18, :], xt[:, 11:18, :])
                nc.scalar.dma_start(x_sb[:, 18:24, :], xt[:, 18:24, :])
                nc.sync.dma_start(x_sb[:, 24:28, :], xt[:, 24:28, :])
                nc.scalar.dma_start(wv_sb[:], wv[:])

                wq_sb = wp.tile([128, REP, DK, 128], BF16, tag="wqt")
                # rope tables after the x odds on the scalar queue (first
                # rope isn't until ~40us in)
                nc.scalar.dma_start(sin_sb[:], sincat[:])
                nc.scalar.dma_start(cos_sb[:], coscat[:])

                def rope(X_full, n):
                    # X = X*cos + swap(X)*[-sin;sin], fp32 intermediates,
                    # one bf16 rounding on the final add
                    X = X_full[:, 512 * n:512 * (n + 1)]
                    ssl = (slice(None), slice(512 * n, 512 * (n + 1)))
                    tA = ropep.tile([128, 512], FP32, tag="ropeA")
                    nc.vector.tensor_copy(tA[0:64, :], X[64:128, :])
                    nc.vector.tensor_copy(tA[64:128, :], X[0:64, :])
                    nc.vector.tensor_tensor(tA[:], tA[:], sin_sb[ssl], op=mult)
                    tB = ropep.tile([128, 512], FP32, tag="ropeB")
                    nc.vector.tensor_tensor(tB[:], X, cos_sb[ssl], op=mult)
                    nc.vector.tensor_tensor(X, tA[:], tB[:], op=addop)

                # m: 0 = k, 1 = v, 2.. = q heads 0..6
                for m in range(REP + 2):
                    if m < REP:
                        # just-in-time wq head DMA (used at iteration m+2):
                        # staggered so it never contends with the x stream
                        nc.gpsimd.dma_start(wq_sb[:, m], wq[:, m])
                    wt = (wk_sb if m == 0 else wv_sb if m == 1
                          else wq_sb[:, m - 2])
                    for n in range(NT):
                        ps = pp1.tile([128, 512], FP32, tag="proj",
                                      name=f"ps_{m}_{n}")
                        for kc in range(DK):
                            nc.tensor.matmul(
                                ps[:],
                                wt[:, kc, :],
                                x_sb[:, kc, 512 * n:512 * (n + 1)],
                                start=(kc == 0),
                                stop=(kc == DK - 1),
                            )
                        if m == 0:
                            dst, bi = k_sb[:, 512 * n:512 * (n + 1)], 7
                        elif m == 1:
                            dst, bi = v_sb[:, 512 * n:512 * (n + 1)], 8
                        else:
                            dst, bi = q_sb[:, m - 2, 512 * n:512 * (n + 1)], m - 2
                        nc.scalar.activation(
                            dst, ps[:], Ident, bias=bias_sb[:, bi:bi + 1],
                            scale=1.0,
                        )
                        if m == 0:
                            rope(k_sb, n)
                        elif m == 1:
                            # v^T chunk -> v natural while q projs stream
                            for sc in range(4 * n, 4 * n + 4):
                                tp = pp1.tile([128, 128], BF16, tag="tr",
                                              name=f"tr_{sc}")
                                nc.tensor.transpose(
                                    tp[:], v_sb[:, 128 * sc:128 * (sc + 1)],
                                    id_sb[:],
                                )
                                nc.scalar.copy(vn_sb[:, sc, :], tp[:])
                        else:
                            rope(q_sb[:, m - 2, :], n)

            # ---- Phase 4: attention per head + pipelined AllGather ------
            otp_ctx = tc.tile_pool(name="otp", bufs=1)
            otp = otp_ctx.__enter__()
            otf = otp.tile([128, DK, T], BF16, tag="otf")
            # o_proj weights stream in during attention (DMA idle then);
            # single contiguous DMA on the vector queue keeps the sync
            # queue free for the og stores feeding the AllGathers
            wp2_ctx = tc.tile_pool(name="wp2", bufs=1)
            wp2 = wp2_ctx.__enter__()
            wo_sb = wp2.tile([128, DK, REP * 128], BF16, tag="wo")
            nc.scalar.dma_start(wo_sb[:], wo[:])
            ppatt_ctx = tc.tile_pool(name="ppatt", bufs=1, space="PSUM")
            ppatt = ppatt_ctx.__enter__()
            pending = []

            def finalize(h, tau, den, ops):
                rec = ep.tile([1, 512], FP32, tag="rec", name=f"rec_{h}_{tau}")
                nc.vector.reciprocal_approx_fast(rec[:], den[0:1, :])
                recb = ep.tile([1, 512], BF16, tag="recb", name=f"recb_{h}_{tau}")
                nc.scalar.copy(recb[:], rec[:])
                bc = ppatt.tile([128, 512], FP32, tag=f"den{tau % 2}",
                                name=f"bc_{h}_{tau}")
                nc.tensor.matmul(bc[:], ones_row[:], recb[:], start=True, stop=True)
                bcs = ep.tile([128, 512], BF16, tag="bcs", name=f"bcs_{h}_{tau}")
                nc.scalar.copy(bcs[:], bc[:])
                ost = ep.tile([128, 512], BF16, tag="ost", name=f"ost_{h}_{tau}")
                nc.vector.tensor_tensor(ost[:], ops[:], bcs[:], op=mult)
                grp = next(i for i, (lo, hi) in enumerate(AGH) if lo <= h < hi)
                lo, hi = AGH[grp]
                nc.sync.dma_start(
                    og[grp][128 * (h - lo):128 * (h - lo + 1),
                            512 * tau:512 * (tau + 1)],
                    ost[:],
                )
                if tau == NT - 1 and h == hi - 1:
                    # trigger only -- the oag->otf gathers are issued after
                    # the attention loop so later AG triggers on this queue
                    # are never blocked behind a wave's completion wait
                    nc.gpsimd.collective_compute(
                        "AllGather",
                        mybir.AluOpType.bypass,
                        replica_groups=RG,
                        ins=[og[grp][:].opt()],
                        outs=[oag[grp][:].opt()],
                    )

            for h in range(REP):
                for tau in range(NT):
                    n_sc = 4 * (tau + 1)
                    den = ppatt.tile([1, 512], FP32, tag=f"den{tau % 2}",
                                     name=f"den_{h}_{tau}")
                    ops = ppatt.tile([128, 512], FP32, tag=f"opv{tau % 2}",
                                     name=f"ops_{h}_{tau}")
                    etiles = {}

                    def emit_s(c):
                        delta = 128 * c - 512 * tau
                        t0 = max(delta, 0)
                        w = 512 - t0
                        sps = ppatt.tile([128, 512], FP32, tag=f"s{c % 4}",
                                         name=f"sps_{h}_{tau}_{c}")
                        tsl = slice(512 * tau + t0, 512 * (tau + 1))
                        nc.tensor.matmul(
                            sps[:, 0:w],
                            k_sb[:, 128 * c:128 * (c + 1)],
                            q_sb[:, h, tsl],
                            start=True,
                            stop=True,
                        )
                        if delta >= 0:
                            nc.vector.tensor_tensor(
                                sps[:, 0:128], sps[:, 0:128], tri_sb[:], op=addop
                            )
                        et = ep.tile([128, 512], BF16, tag="e",
                                     name=f"et_{h}_{tau}_{c}")
                        nc.scalar.activation(et[:, 0:w], sps[:, 0:w], Exp, scale=SCALE)
                        etiles[c] = (et, t0, w)

                    def emit_acc(c):
                        et, t0, w = etiles.pop(c)
                        nc.tensor.matmul(
                            den[0:1, t0:512], ones_col[:], et[:, 0:w],
                            start=(c == 0), stop=(c == n_sc - 1),
                        )
                        nc.tensor.matmul(
                            ops[:, t0:512], vn_sb[:, c, :], et[:, 0:w],
                            start=(c == 0), stop=(c == n_sc - 1),
                        )

                    LOOKAHEAD = 3
                    for c in range(n_sc):
                        emit_s(c)
                        if c == LOOKAHEAD and pending:
                            finalize(*pending.pop(0))
                        if c >= LOOKAHEAD:
                            emit_acc(c - LOOKAHEAD)
                    for c in range(max(0, n_sc - LOOKAHEAD), n_sc):
                        emit_acc(c)
                    pending.append((h, tau, den, ops))

            while pending:
                finalize(*pending.pop(0))
            ppatt_ctx.__exit__(None, None, None)

            # oag -> otf gathers, wave-major on the sync queue: wave w's
            # copies wait only on AG_w, and the o_proj consumer reads otf
            # in the same wave order
            for grp, (lo, hi) in enumerate(AGH):
                nh = hi - lo
                for hh in range(lo, hi):
                    for gp in range(GROUP):
                        r0 = nh * 128 * gp + 128 * (hh - lo)
                        nc.sync.dma_start(
                            otf[:, 7 * gp + hh, :],
                            oag[grp][r0:r0 + 128, :],
                        )

            # ---- Phase 6: o_proj as per-AG-wave partial sums ------------
            # Each AG wave's contribution is an independent PSUM group,
            # added into an SBUF accumulator on the DVE as soon as the wave
            # lands; token tiles paired per stationary weight slice.
            with (
                tc.tile_pool(name="yaccp", bufs=1) as yaccp,
                tc.tile_pool(name="pp6", bufs=4, space="PSUM") as pp6,
            ):
                yacc = yaccp.tile([128, REP, T], FP32, tag="yacc")
                ytr = yt.rearrange("(m p) t -> p m t", p=128)
                for wi, (lo, hi) in enumerate(AGH):
                    hgs = [7 * gp + hh for hh in range(lo, hi)
                           for gp in range(GROUP)]
                    for m in range(REP):
                        for n in range(NT):
                            ps = pp6.tile([128, 512], FP32, tag="y",
                                          name=f"y_{wi}_{m}_{n}")
                            for j, hg in enumerate(hgs):
                                nc.tensor.matmul(
                                    ps[:],
                                    wo_sb[:, hg, 128 * m:128 * (m + 1)],
                                    otf[:, hg, 512 * n:512 * (n + 1)],
                                    start=(j == 0),
                                    stop=(j == len(hgs) - 1),
                                )
                            dst = yacc[:, m, 512 * n:512 * (n + 1)]
                            if wi == 0:
                                nc.scalar.copy(dst, ps[:])
                            else:
                                nc.vector.tensor_tensor(dst, dst, ps[:],
                                                        op=addop)
                        if wi == len(AGH) - 1:
                            nc.sync.dma_start(ytr[:, m, :], yacc[:, m, :])
            wp2_ctx.__exit__(None, None, None)
            otp_ctx.__exit__(None, None, None)

    nc.compile()
    return nc


def _host_prep(x, segment_ids, Wq, bq, Wk, bk, Wv, bv, Wo):
    """Numpy-side input prep: transpose x, slice weights, RoPE tables, mask."""
    valid = (segment_ids != 0)
    pos = (np.cumsum(valid, axis=-1) - 1).astype(np.int32)  # CUR_IND = 0
    half = HEAD_DIM // 2
    fraction = np.arange(half, dtype=np.float32) / half
    timescale = ROPE_THETA ** fraction
    ang = pos[..., None].astype(np.float32) / timescale      # (B, T, 64)
    sin = np.sin(ang).astype(np.float32)
    cos = np.cos(ang).astype(np.float32)

    sl = np.arange(128)
    tri = np.where(sl[None, :] >= sl[:, None], 0.0, K_MASK).astype(np.float32)

    BF = ml_dtypes.bfloat16
    # pre-transpose weights to [partition, chunk, out] so each kernel DMA is
    # one contiguous run per partition: W[c*128+p, n] -> [p, c, n]
    Wq_p = Wq.reshape(DK, 128, NUM_HEADS * HEAD_DIM).transpose(1, 0, 2)
    Wk_p = Wk.reshape(DK, 128, 512).transpose(1, 0, 2)
    Wv_p = Wv.reshape(DK, 128, 512).transpose(1, 0, 2)
    Wo_p = Wo.reshape(DK, 128, D).transpose(1, 0, 2)
    in_maps = []
    for c in range(NCORES):
        b, g = c // GROUP, c % GROUP
        qcols = slice(REP * 128 * g, REP * 128 * (g + 1))
        kvcols = slice(128 * g, 128 * (g + 1))
        bias = np.concatenate(
            [bq[qcols].reshape(REP, 128), bk[kvcols][None, :], bv[kvcols][None, :]],
            axis=0,
        ).astype(np.float32)
        sincat = np.concatenate([-sin[b].T, sin[b].T], axis=0)  # (128, T)
        coscat = np.concatenate([cos[b].T, cos[b].T], axis=0)
        # wq as [p, head, chunk, 128]
        wq_core = (Wq_p[:, :, qcols].reshape(128, DK, REP, 128)
                   .transpose(0, 2, 1, 3))
        xp = x[b].T.reshape(DK, 128, T).transpose(1, 0, 2)  # [p, c, t]
        in_maps.append({
            "xt": np.ascontiguousarray(xp).astype(BF),
            "wq": np.ascontiguousarray(wq_core).astype(BF),
            "wk": np.ascontiguousarray(Wk_p[:, :, kvcols]).astype(BF),
            "wv": np.ascontiguousarray(Wv_p[:, :, kvcols]).astype(BF),
            "wo": np.ascontiguousarray(Wo_p[:, :, qcols]).astype(BF),
            "bqkv": bias,
            "sincat": np.ascontiguousarray(sincat, dtype=np.float32),
            "coscat": np.ascontiguousarray(coscat, dtype=np.float32),
            "trimask": tri,
            "onescol": np.ones((128, 1), BF),
            "onesrow": np.ones((1, 128), BF),
        })
    return in_maps


def _assemble(results):
    y = np.empty((B, T, D), dtype=np.float32)
    for b in range(B):
        blocks = [results[GROUP * b + g]["yt"] for g in range(GROUP)]
        y[b] = np.concatenate(blocks, axis=0).T
    return y


def kernel(x, segment_ids, k_cache, v_cache, Wq, bq, Wk, bk, Wv, bv, Wo,
           _trace=False, _trace_kwargs=None):
    # k_cache/v_cache are zero-initialized and fully overwritten by this
    # prefill (CUR_IND=0, cache_size==T), so they do not affect the output.
    from concourse.bass_utils import run_bass_kernel_spmd

    in_maps = _host_prep(
        np.asarray(x), np.asarray(segment_ids),
        np.asarray(Wq), np.asarray(bq), np.asarray(Wk), np.asarray(bk),
        np.asarray(Wv), np.asarray(bv), np.asarray(Wo),
    )
    if "nc" not in _CACHE:
        _CACHE["nc"] = _build_nc()
    kw = {}
    if _trace:
        kw.update(trace=True, **(_trace_kwargs or {}))
    br = run_bass_kernel_spmd(_CACHE["nc"], in_maps, core_ids=list(range(NCORES)), **kw)
    y = _assemble(br.results)
    if _trace:
        _CACHE["last_result"] = br
    return y
